# revision 17
# baseline (speedup 1.0000x reference)
"""Trainium2 Bass kernel for nn_Classifier_59270548685016.

Computes prediction[b, k] = sum(x[b] * classification_images[k]) — i.e. a
[256, 150528] x [150528, 1000] matmul producing [256, 1000] f32.

Strategy:
- Shard the contraction dim D = 3*224*224 = 150528 across the 8 NeuronCores
  (18816 each). Each core computes a partial [256, 1000] f32 product over its
  D-slice; the host sums the 8 partials. Every input byte is read exactly once
  (minimal HBM traffic for any sharding): ~47 MB/core fp16 => ~131 us DMA
  floor vs ~122 us PE floor per core — near-balanced, compute regime.
- Operands are cast to fp16 on the host (randn data: no overflow, ~2^-11
  relative rounding). The PE runs fp16 at the full 1 cycle/row rate and
  accumulates in fp32 PSUM => measured rel err ~2.4e-4.
- Inputs are pre-swizzled on the host into the exact SBUF image layout
  ([128 partitions, k-tile-major free dim]) so every DMA is a plain
  contiguous-per-partition copy at full bandwidth.
- Per core: 147 contraction k-tiles of 128. x^T (9.6 MB) is fully resident in
  SBUF (loaded in 21 chunks so PE starts early); c^T streams through a
  16-deep pool of 3-k-tile chunks (750 KB SWDGE DMAs). Output split
  2x(m=128) x 2x(n=500) PSUM banks; 588 accumulating matmuls, one drain.
- Must be built with bacc.Bacc + nc.compile(): bacc legalizes instructions
  carrying >1 semaphore wait (walrus rejects those — "Too many sync wait
  commands"), which every slot-reuse DMA and the kernel-tail drain need.
"""

import numpy as np

import concourse.bacc as bacc_mod
import concourse.mybir as mybir
import concourse.tile as tile
from concourse.bass_utils import run_bass_kernel_spmd

B = 256                 # batch (output rows)
K = 1000                # classes (output cols)
D = 3 * 224 * 224       # contraction dim, 150528
NCORES = 8
P = 128                 # partitions / PE contraction tile
DC = D // NCORES        # 18816 per-core contraction slice
KT = DC // P            # 147 contraction k-tiles per core
NSPLIT = 2
NTILE = K // NSPLIT     # 500 columns: fits one PSUM bank (<=512 f32)
MT = B // P             # 2 output-row tiles

_CACHE: dict = {}


def _build(kt=KT, kchunk=3, b=B, k=K, ct_bufs=16, xt_chunk=7):
    """Build the per-core module (same program on all 8 cores)."""
    mt = b // P
    ntile = k // NSPLIT
    nchunk = kt // kchunk
    assert kt % kchunk == 0 and kt % xt_chunk == 0 and b % P == 0

    nc = bacc_mod.Bacc("TRN2", debug=False, num_devices=NCORES)
    xt_in = nc.dram_tensor("xt", [P, kt * b], mybir.dt.float16, kind="ExternalInput").ap()
    ct_in = nc.dram_tensor("ct", [P, kt * k], mybir.dt.float16, kind="ExternalInput").ap()
    out = nc.dram_tensor("out", [b, k], mybir.dt.float32, kind="ExternalOutput").ap()

    with tile.TileContext(nc) as tc:
        with (
            tc.tile_pool(name="xtp", bufs=1) as xt_pool,
            tc.tile_pool(name="ctp", bufs=ct_bufs) as ct_pool,
            tc.tile_pool(name="otp", bufs=1) as out_pool,
            tc.tile_pool(name="psp", bufs=1, space="PSUM") as psum_pool,
        ):
            psums = [
                [
                    psum_pool.tile([P, ntile], mybir.dt.float32, tag=f"ps{m}_{n}", name=f"ps{m}_{n}")
                    for n in range(NSPLIT)
                ]
                for m in range(mt)
            ]
            xts = []
            for j in range(kt // xt_chunk):
                xt_sb = xt_pool.tile([P, xt_chunk * b], mybir.dt.float16, tag=f"xt{j}", name=f"xt{j}")
                nc.sync.dma_start(xt_sb, xt_in[:, j * xt_chunk * b:(j + 1) * xt_chunk * b])
                xts.append(xt_sb)

            for j in range(nchunk):
                ct_sb = ct_pool.tile([P, kchunk * k], mybir.dt.float16, tag="ct", name=f"ct{j}")
                nc.gpsimd.dma_start(ct_sb, ct_in[:, j * kchunk * k:(j + 1) * kchunk * k])
                for kk in range(kchunk):
                    ki = j * kchunk + kk
                    xj, xk = divmod(ki, xt_chunk)
                    for m in range(mt):
                        lhsT = xts[xj][:, xk * b + m * P: xk * b + (m + 1) * P]
                        for n in range(NSPLIT):
                            nc.tensor.matmul(
                                psums[m][n],
                                lhsT,
                                ct_sb[:, kk * k + n * ntile: kk * k + (n + 1) * ntile],
                                start=(ki == 0),
                                stop=(ki == kt - 1),
                            )

            for m in range(mt):
                ot = out_pool.tile([P, k], mybir.dt.float32, tag=f"ot{m}", name=f"ot{m}")
                for n in range(NSPLIT):
                    nc.vector.tensor_copy(ot[:, n * ntile:(n + 1) * ntile], psums[m][n])
                nc.sync.dma_start(out[m * P:(m + 1) * P, :], ot)

    nc.compile()
    return nc


def _get_nc():
    if "nc" not in _CACHE:
        _CACHE["nc"] = _build()
    return _CACHE["nc"]


def _prep_in_maps(x, classification_images):
    """Cast to fp16 and swizzle shards into the SBUF image layout.

    xt[p, ki*B + m] = x[m, i*DC + ki*P + p];  ct[p, ki*K + n] = c[n, i*DC + ki*P + p].
    """
    x_flat = np.asarray(x).reshape(B, D).astype(np.float16)
    c_flat = np.asarray(classification_images).reshape(K, D).astype(np.float16)
    in_maps = []
    for i in range(NCORES):
        sl = slice(i * DC, (i + 1) * DC)
        xt = x_flat[:, sl].T.reshape(KT, P, B).transpose(1, 0, 2).reshape(P, KT * B)
        ct = c_flat[:, sl].T.reshape(KT, P, K).transpose(1, 0, 2).reshape(P, KT * K)
        in_maps.append(
            {"xt": np.ascontiguousarray(xt), "ct": np.ascontiguousarray(ct)}
        )
    return in_maps


def _run(in_maps, **kwargs):
    return run_bass_kernel_spmd(_get_nc(), in_maps, core_ids=list(range(NCORES)), **kwargs)


def kernel(x, classification_images):
    in_maps = _prep_in_maps(x, classification_images)
    res = _run(in_maps)
    out = np.zeros((B, K), np.float32)
    for r in res.results:
        out += r["out"]
    return (out,)


# revision 18
# speedup vs baseline: 1.0640x; 1.0640x over previous
"""Trainium2 Bass kernel for nn_Classifier_59270548685016.

Computes prediction[b, k] = sum(x[b] * classification_images[k]) — i.e. a
[256, 150528] x [150528, 1000] matmul producing [256, 1000] f32.

Strategy:
- Shard the contraction dim D = 3*224*224 = 150528 across the 8 NeuronCores
  (18816 each). Each core computes a partial [256, 1000] f32 product over its
  D-slice; the host sums the 8 partials. Every input byte is read exactly once
  (minimal HBM traffic for any sharding): ~47 MB/core fp16 => ~131 us DMA
  floor vs ~122 us PE floor per core — near-balanced, compute regime.
- Operands are cast to fp16 on the host (randn data: no overflow, ~2^-11
  relative rounding). The PE runs fp16 at the full 1 cycle/row rate and
  accumulates in fp32 PSUM => measured rel err ~2.4e-4.
- Inputs are pre-swizzled on the host into the exact SBUF image layout
  ([128 partitions, k-tile-major free dim]) so every DMA is a plain
  contiguous-per-partition copy at full bandwidth.
- Per core: 147 contraction k-tiles of 128. x^T (9.6 MB) is fully resident in
  SBUF (loaded in 49 chunks so PE starts early); c^T streams through a
  16-deep pool of 3-k-tile chunks (750 KB SWDGE DMAs). Output split
  2x(m=128) x 2x(n=500) PSUM banks; 588 accumulating matmuls, one drain.
- Must be built with bacc.Bacc + nc.compile(): bacc legalizes instructions
  carrying >1 semaphore wait (walrus rejects those — "Too many sync wait
  commands"), which every slot-reuse DMA and the kernel-tail drain need.
"""

import numpy as np

import concourse.bacc as bacc_mod
import concourse.mybir as mybir
import concourse.tile as tile
from concourse.bass_utils import run_bass_kernel_spmd

B = 256                 # batch (output rows)
K = 1000                # classes (output cols)
D = 3 * 224 * 224       # contraction dim, 150528
NCORES = 8
P = 128                 # partitions / PE contraction tile
DC = D // NCORES        # 18816 per-core contraction slice
KT = DC // P            # 147 contraction k-tiles per core
NSPLIT = 2
NTILE = K // NSPLIT     # 500 columns: fits one PSUM bank (<=512 f32)
MT = B // P             # 2 output-row tiles

_CACHE: dict = {}


def _build(kt=KT, kchunk=3, b=B, k=K, ct_bufs=16, xt_chunk=3):
    """Build the per-core module (same program on all 8 cores)."""
    mt = b // P
    ntile = k // NSPLIT
    nchunk = kt // kchunk
    assert kt % kchunk == 0 and kt % xt_chunk == 0 and b % P == 0

    nc = bacc_mod.Bacc("TRN2", debug=False, num_devices=NCORES)
    xt_in = nc.dram_tensor("xt", [P, kt * b], mybir.dt.float16, kind="ExternalInput").ap()
    ct_in = nc.dram_tensor("ct", [P, kt * k], mybir.dt.float16, kind="ExternalInput").ap()
    out = nc.dram_tensor("out", [b, k], mybir.dt.float32, kind="ExternalOutput").ap()

    with tile.TileContext(nc) as tc:
        with (
            tc.tile_pool(name="xtp", bufs=1) as xt_pool,
            tc.tile_pool(name="ctp", bufs=ct_bufs) as ct_pool,
            tc.tile_pool(name="otp", bufs=1) as out_pool,
            tc.tile_pool(name="psp", bufs=1, space="PSUM") as psum_pool,
        ):
            psums = [
                [
                    psum_pool.tile([P, ntile], mybir.dt.float32, tag=f"ps{m}_{n}", name=f"ps{m}_{n}")
                    for n in range(NSPLIT)
                ]
                for m in range(mt)
            ]
            xts = []
            for j in range(kt // xt_chunk):
                xt_sb = xt_pool.tile([P, xt_chunk * b], mybir.dt.float16, tag=f"xt{j}", name=f"xt{j}")
                nc.sync.dma_start(xt_sb, xt_in[:, j * xt_chunk * b:(j + 1) * xt_chunk * b])
                xts.append(xt_sb)

            for j in range(nchunk):
                ct_sb = ct_pool.tile([P, kchunk * k], mybir.dt.float16, tag="ct", name=f"ct{j}")
                nc.gpsimd.dma_start(ct_sb, ct_in[:, j * kchunk * k:(j + 1) * kchunk * k])
                for kk in range(kchunk):
                    ki = j * kchunk + kk
                    xj, xk = divmod(ki, xt_chunk)
                    for m in range(mt):
                        lhsT = xts[xj][:, xk * b + m * P: xk * b + (m + 1) * P]
                        for n in range(NSPLIT):
                            nc.tensor.matmul(
                                psums[m][n],
                                lhsT,
                                ct_sb[:, kk * k + n * ntile: kk * k + (n + 1) * ntile],
                                start=(ki == 0),
                                stop=(ki == kt - 1),
                            )

            for m in range(mt):
                ot = out_pool.tile([P, k], mybir.dt.float32, tag=f"ot{m}", name=f"ot{m}")
                for n in range(NSPLIT):
                    nc.vector.tensor_copy(ot[:, n * ntile:(n + 1) * ntile], psums[m][n])
                nc.sync.dma_start(out[m * P:(m + 1) * P, :], ot)

    nc.compile()
    return nc


def _get_nc():
    if "nc" not in _CACHE:
        _CACHE["nc"] = _build()
    return _CACHE["nc"]


def _prep_in_maps(x, classification_images):
    """Cast to fp16 and swizzle shards into the SBUF image layout.

    xt[p, ki*B + m] = x[m, i*DC + ki*P + p];  ct[p, ki*K + n] = c[n, i*DC + ki*P + p].
    """
    x_flat = np.asarray(x).reshape(B, D).astype(np.float16)
    c_flat = np.asarray(classification_images).reshape(K, D).astype(np.float16)
    in_maps = []
    for i in range(NCORES):
        sl = slice(i * DC, (i + 1) * DC)
        xt = x_flat[:, sl].T.reshape(KT, P, B).transpose(1, 0, 2).reshape(P, KT * B)
        ct = c_flat[:, sl].T.reshape(KT, P, K).transpose(1, 0, 2).reshape(P, KT * K)
        in_maps.append(
            {"xt": np.ascontiguousarray(xt), "ct": np.ascontiguousarray(ct)}
        )
    return in_maps


def _run(in_maps, **kwargs):
    return run_bass_kernel_spmd(_get_nc(), in_maps, core_ids=list(range(NCORES)), **kwargs)


def kernel(x, classification_images):
    in_maps = _prep_in_maps(x, classification_images)
    res = _run(in_maps)
    out = np.zeros((B, K), np.float32)
    for r in res.results:
        out += r["out"]
    return (out,)
